# revision 1
# baseline (speedup 1.0000x reference)
"""Single-head causal attention (B=4, T=4096, E=1024, H=128) on 8 trn2 cores.

Sharding: core c -> (batch b = c//2, piece p = c%2). Within a batch the 32
query blocks of 128 rows are split even/odd between the two pieces so the
causal workload balances. The device program is identical on all cores
(SPMD); all per-core differences are carried by the input data (gathered
query rows + a causal-boundary mask strip).

Device algorithm (per core, all "transposed" layouts):
  QT = Wq @ xq^T           [H=128, 2048]   (gathered query rows)
  KT = Wk @ x^T            [H=128, 4096]
  VT = Wv @ x^T  -> PE-transpose -> V blocks [128 tok, 128 h]
  per q-tile K (512 queries = in-tile blocks i=0..3):
    for kb in 0..8K+7:   ST[kb] = KT_blk^T @ QT_tile   [128 k, 512 q] (PSUM)
      last 8 kb get an additive causal mask strip (per-core data)
      PT = exp(scale * ST)                             (ACT, PSUM->SBUF)
      OT += V_blk^T @ PT    [128 h, 512 q]             (PSUM accum)
      l  += ones^T @ PT     [1, 512 q]                 (PSUM accum)
    O = (OT / l)^T via PE transpose + per-partition scalar multiply
Matmuls run as float32r (full PE rate for free dim >= 256; fp32 data bits).
"""

import os
import numpy as np

B, T, E, H = 4, 4096, 1024, 128
P = 128
NB_E = E // P           # 8 contraction chunks
TQ = T // 2             # 2048 gathered queries per core
N_QT = TQ // 512        # 4 q-tiles per core
SCALE = float(H) ** -0.5
NEG = -30000.0
N_CORES = 8
F32 = np.float32


def _query_rows(p: int) -> np.ndarray:
    """Absolute row indices of the gathered queries for piece p (in order)."""
    blocks = [np.arange(256 * g + 128 * p, 256 * g + 128 * p + 128) for g in range(16)]
    return np.concatenate(blocks)


def _mask_strip(p: int) -> np.ndarray:
    """maskT [1024 k, 512 q]: 0 where key visible, NEG where masked.

    Row 128*j + kk is in-strip key block j (j=0..7); col 128*i + r is
    in-tile query block i. Visible iff 128*j + kk <= 256*i + 128*p + r.
    """
    kk = np.arange(1024)[:, None]           # 128*j + kk
    qq = np.arange(512)[None, :]
    i, r = qq // 128, qq % 128
    visible = kk <= 256 * i + 128 * p + r
    return np.where(visible, 0.0, NEG).astype(F32)


def _emit(tc, aps):
    import concourse.bass as bass
    from concourse import mybir
    from concourse.masks import make_identity

    nc = tc.nc
    f32 = mybir.dt.float32
    f16 = mybir.dt.float16
    EXP = mybir.ActivationFunctionType.Exp

    xT, xqT, wq, wk, wv, maskT, out = aps

    from contextlib import ExitStack

    ctx = ExitStack()
    with ctx:
        # ---- pools ----
        consts = ctx.enter_context(tc.tile_pool(name="consts", bufs=1))
        x_pool = ctx.enter_context(tc.tile_pool(name="x", bufs=96))
        vt_pool = ctx.enter_context(tc.tile_pool(name="vt", bufs=2))
        pt_pool = ctx.enter_context(tc.tile_pool(name="pt", bufs=4))
        osb_pool = ctx.enter_context(tc.tile_pool(name="osb", bufs=2))
        on_pool = ctx.enter_context(tc.tile_pool(name="on", bufs=4))
        sm_pool = ctx.enter_context(tc.tile_pool(name="sm", bufs=4))
        s_ps = ctx.enter_context(tc.tile_pool(name="sps", bufs=3, space="PSUM"))
        o_ps = ctx.enter_context(tc.tile_pool(name="ops", bufs=2, space="PSUM"))
        l_ps = ctx.enter_context(tc.tile_pool(name="lps", bufs=1, space="PSUM"))
        t_ps = ctx.enter_context(tc.tile_pool(name="tps", bufs=2, space="PSUM"))

        # ---- persistent SBUF tensors ----
        identity = consts.tile([P, P], f32)
        ones = consts.tile([P, 1], f16)
        ones32 = consts.tile([1, 1], f32)
        wq_sb = consts.tile([P, NB_E, P], f16)
        wk_sb = consts.tile([P, NB_E, P], f16)
        wv_sb = consts.tile([P, NB_E, P], f16)
        mask_sb = consts.tile([P, 8, 512], f32)
        kt_all = consts.tile([P, T], f16)
        v_all = consts.tile([P, T // P, P], f16)
        qt_all = consts.tile([P, TQ], f16)

        make_identity(nc, identity[:])
        nc.gpsimd.memset(ones[:], 1.0)
        nc.gpsimd.memset(ones32[:], 1.0)
        nc.sync.dma_start(wq_sb[:], wq.rearrange("(c p) h -> p c h", p=P))
        nc.sync.dma_start(wk_sb[:], wk.rearrange("(c p) h -> p c h", p=P))
        nc.sync.dma_start(wv_sb[:], wv.rearrange("(c p) h -> p c h", p=P))
        nc.sync.dma_start(mask_sb[:], maskT.rearrange("(j p) q -> p j q", p=P))

        def load_x_tiles(src_ap, t0):
            tiles = []
            for c in range(NB_E):
                xt = x_pool.tile([P, 512], f16, tag="x")
                nc.sync.dma_start(xt[:], src_ap[c * P:(c + 1) * P, t0:t0 + 512])
                tiles.append(xt)
            return tiles

        def project(w_sb, x_tiles, dst_ap):
            ps = s_ps.tile([P, 512], f32, tag="sps")
            for c in range(NB_E):
                nc.tensor.matmul(
                    ps[:],
                    lhsT=w_sb[:, c, :],
                    rhs=x_tiles[c][:],
                    start=(c == 0),
                    stop=(c == NB_E - 1),
                )
            nc.vector.tensor_copy(dst_ap, ps[:])
            return ps

        # ---- rounds: interleave projections with attention q-tiles ----
        for tt in range(N_QT):
            # Q projection for q-tile tt
            xq_tiles = load_x_tiles(xqT, tt * 512)
            project(wq_sb, xq_tiles, qt_all[:, tt * 512:(tt + 1) * 512])

            # K/V projections for token tiles 2tt, 2tt+1
            for tok in (2 * tt, 2 * tt + 1):
                xk_tiles = load_x_tiles(xT, tok * 512)
                project(wk_sb, xk_tiles, kt_all[:, tok * 512:(tok + 1) * 512])
                vt = vt_pool.tile([P, 512], f32, tag="vt")
                project(wv_sb, xk_tiles, vt[:])
                for u in range(4):
                    kb = tok * 4 + u
                    tp = t_ps.tile([P, P], f32, tag="tps")
                    nc.tensor.transpose(tp[:], vt[:, u * P:(u + 1) * P], identity[:])
                    nc.vector.tensor_copy(v_all[:, kb, :], tp[:])

            # attention for q-tile tt
            qs = qt_all[:, tt * 512:(tt + 1) * 512]
            ot = o_ps.tile([P, 512], f32, tag="ops")
            lt = l_ps.tile([1, 512], f32, tag="lps")
            nkb = 8 * tt + 8

            s_tiles = [None] * nkb

            def emit_scores(kb):
                if kb < 8 * tt:
                    c0 = 0
                else:
                    j = kb - 8 * tt
                    c0 = P * max(0, -(-(128 * j - 255) // 256))
                s = s_ps.tile([P, 512], f32, tag="sps", name=f"s_{tt}_{kb}")
                nc.tensor.matmul(
                    s[:, c0:512],
                    lhsT=kt_all[:, kb * P:(kb + 1) * P],
                    rhs=qs[:, c0:512],
                    start=True,
                    stop=True,
                )
                s_tiles[kb] = s

            def c0_of(kb):
                if kb < 8 * tt:
                    return 0
                j = kb - 8 * tt
                return P * max(0, -(-(128 * j - 255) // 256))

            emit_scores(0)
            for kb in range(nkb):
                if kb + 1 < nkb:
                    emit_scores(kb + 1)
                s = s_tiles[kb]
                c0 = c0_of(kb)
                if kb >= 8 * tt:
                    # the causal boundary lives in a single 128-col block
                    # (= block c0//128); mask is 0 everywhere right of it
                    j = kb - 8 * tt
                    nc.vector.tensor_add(
                        s[:, c0:c0 + P], s[:, c0:c0 + P],
                        mask_sb[:, j, c0:c0 + P])
                pt = pt_pool.tile([P, 512], f16, tag="pt")
                nc.scalar.activation(pt[:, c0:512], s[:, c0:512], EXP, scale=SCALE)
                nc.tensor.matmul(
                    ot[:, c0:512],
                    lhsT=v_all[:, kb, :],
                    rhs=pt[:, c0:512],
                    start=(kb == 0),
                    stop=(kb == nkb - 1),
                )
                nc.tensor.matmul(
                    lt[:1, c0:512],
                    lhsT=ones[:],
                    rhs=pt[:, c0:512],
                    start=(kb == 0),
                    stop=(kb == nkb - 1),
                )

            # epilogue: normalize + transpose + store
            o_sb = osb_pool.tile([P, 512], f32, tag="osb")
            nc.vector.tensor_copy(o_sb[:], ot[:])
            l_sb = sm_pool.tile([1, 512], f32, tag="lsb")
            nc.vector.tensor_copy(l_sb[:], lt[:])
            on = on_pool.tile([P, 4, P], f32, tag="on")
            for i in range(4):
                lc = t_ps.tile([P, 1], f32, tag="tps", name=f"lc_{tt}_{i}")
                nc.tensor.matmul(
                    lc[:],
                    lhsT=l_sb[:1, i * P:(i + 1) * P],
                    rhs=ones32[:],
                    start=True,
                    stop=True,
                )
                rlc = sm_pool.tile([P, 1], f32, tag="rlc")
                nc.vector.reciprocal(rlc[:], lc[:])
                tp = t_ps.tile([P, P], f32, tag="tps", name=f"otp_{tt}_{i}")
                nc.tensor.transpose(tp[:], o_sb[:, i * P:(i + 1) * P], identity[:])
                nc.vector.tensor_scalar_mul(on[:, i, :], tp[:], rlc[:])
            nc.gpsimd.dma_start(
                out[tt * 512:(tt + 1) * 512, :].rearrange("(i p) h -> p i h", p=P),
                on[:],
            )


def build_program():
    import concourse.tile as tile
    from concourse import bacc, mybir

    f32 = mybir.dt.float32
    f16 = mybir.dt.float16
    nc = bacc.Bacc("TRN2", target_bir_lowering=False, debug=False,
                   num_devices=N_CORES)
    xT = nc.dram_tensor("xT", [E, T], f16, kind="ExternalInput").ap()
    xqT = nc.dram_tensor("xqT", [E, TQ], f16, kind="ExternalInput").ap()
    wq = nc.dram_tensor("wq", [E, H], f16, kind="ExternalInput").ap()
    wk = nc.dram_tensor("wk", [E, H], f16, kind="ExternalInput").ap()
    wv = nc.dram_tensor("wv", [E, H], f16, kind="ExternalInput").ap()
    maskT = nc.dram_tensor("maskT", [1024, 512], f32, kind="ExternalInput").ap()
    out = nc.dram_tensor("out", [TQ, H], f32, kind="ExternalOutput").ap()

    with tile.TileContext(nc) as tc:
        _emit(tc, (xT, xqT, wq, wk, wv, maskT, out))
    nc.compile()
    return nc


def make_in_maps(x, Wq, Wk, Wv):
    """Per-core input maps. x: [B,T,E] f32; W*: [H,E] f32."""
    x = np.asarray(x, dtype=F32)
    wq_t = np.ascontiguousarray(np.asarray(Wq, dtype=F32).T.astype(np.float16))
    wk_t = np.ascontiguousarray(np.asarray(Wk, dtype=F32).T.astype(np.float16))
    wv_t = np.ascontiguousarray(np.asarray(Wv, dtype=F32).T.astype(np.float16))
    in_maps = []
    for c in range(N_CORES):
        b, p = c // 2, c % 2
        xb = x[b]                                              # [T, E]
        xT_np = np.ascontiguousarray(xb.T.astype(np.float16))
        xqT_np = np.ascontiguousarray(xb[_query_rows(p)].T.astype(np.float16))
        in_maps.append({
            "xT": xT_np,
            "xqT": xqT_np,
            "wq": wq_t,
            "wk": wk_t,
            "wv": wv_t,
            "maskT": _mask_strip(p),
        })
    return in_maps


def _enable_ldw_opt():
    """Re-enable walrus's LDWEIGHTS optimization (defaults off in this
    toolchain); correctness is covered by the output check."""
    import concourse.bass_utils as bu
    if getattr(bu, "_ldw_patched", False):
        return
    orig = bu.run_command

    def patched(cmd, *a, **kw):
        cmd = list(cmd)
        return orig(cmd, *a, **kw)

    bu.run_command = patched
    bu._ldw_patched = True


def run(x, Wq, Wk, Wv, trace=False, trace_cores=None):
    """Returns (full_output [B,T,H] f32, BassKernelResults)."""
    from concourse.bass_utils import run_bass_kernel_spmd

    _enable_ldw_opt()

    nc = build_program()
    in_maps = make_in_maps(x, Wq, Wk, Wv)
    res = run_bass_kernel_spmd(
        nc, in_maps, list(range(N_CORES)), trace=trace,
        trace_cores=trace_cores,
    )
    full = np.empty((B, T, H), dtype=F32)
    for c in range(N_CORES):
        b, p = c // 2, c % 2
        full[b, _query_rows(p), :] = res.results[c]["out"]
    return full, res


def kernel(x, Wq, Wk, Wv):
    full, _ = run(x, Wq, Wk, Wv, trace=False)
    return full


if __name__ == "__main__":
    # quick smoke: build program only
    nc = build_program()
    print("program built ok")



# revision 7
# speedup vs baseline: 1.1318x; 1.1318x over previous
"""Single-head causal attention (B=4, T=4096, E=1024, H=128) on 8 trn2 cores.

Sharding: core c -> (batch b = c//2, piece p = c%2). Within a batch the 32
query blocks of 128 rows are split even/odd between the two pieces so the
causal workload balances. The device program is identical on all cores
(SPMD); all per-core differences are carried by the input data (a causal
boundary mask strip; queries are gathered on-device from xT).

Device algorithm (per core, "transposed" layouts):
  warmup: 8 dummy matmuls at t~0 ramp the PE p-state during the DMA wait
  QT = Wq @ xq^T           [H=128, 2048]   (query cols gathered from xT)
  KT = Wk @ x^T            [H=128, 4096]
  VT = Wv @ x^T  -> PE-transpose -> V blocks [128 tok, 128 h]
  per q-tile (512 queries): for kb in 0..8tt+7:
    ST[kb] = KT_blk^T @ QT_tile   [128 k, 512 q]  (PSUM)
    diag blocks get an additive causal mask strip (f16, per-core data)
    PT = exp(scale * ST)          (ACT, PSUM->SBUF f16)
    OT += V_blk^T @ PT            [128 h, 512 q]  (PSUM accum)
    acc += PT                     (DVE, SBUF f32) -- replaces the per-kb
                                  ones^T matmul for the softmax denominator
  epilogue per 128-q block i:
    lc_i = acc_blk^T @ ones       [128 q, 1]  (tiny 1-row matmul)
    O_i  = (o_sb_i)^T * (1/lc_i)  (PE f16 transpose + DVE scalar mul)
"""

import numpy as np

B, T, E, H = 4, 4096, 1024, 128
P = 128
NB_E = E // P           # 8 contraction chunks
TQ = T // 2             # 2048 gathered queries per core
N_QT = TQ // 512        # 4 q-tiles per core
SCALE = float(H) ** -0.5
NEG = -30000.0
N_CORES = 8
F32 = np.float32


def _query_rows(p: int) -> np.ndarray:
    """Absolute row indices of the gathered queries for piece p (in order)."""
    blocks = [np.arange(256 * g + 128 * p, 256 * g + 128 * p + 128) for g in range(16)]
    return np.concatenate(blocks)


def _mask_strip(p: int) -> np.ndarray:
    """mask [128 kk, 8 j, 512 q] f16: 0 where key visible, NEG where masked.

    Key row = 128*j + kk (in-strip block j); col 128*i + r is in-tile query
    block i. Visible iff 128*j + kk <= 256*i + 128*p + r.
    """
    kk = np.arange(1024)[:, None]           # 128*j + kk
    qq = np.arange(512)[None, :]
    i, r = qq // 128, qq % 128
    visible = kk <= 256 * i + 128 * p + r
    m = np.where(visible, 0.0, NEG).astype(np.float16)   # [1024, 512]
    return np.ascontiguousarray(m.reshape(8, 128, 512).transpose(1, 0, 2))


def _emit(tc, aps, piece_of_core=None):
    import concourse.bass as bass
    from concourse import mybir
    from concourse.masks import make_identity

    nc = tc.nc
    f32 = mybir.dt.float32
    f16 = mybir.dt.float16
    EXP = mybir.ActivationFunctionType.Exp

    xT, xqT, wq, wk, wv, maskT, out = aps

    from contextlib import ExitStack

    ctx = ExitStack()
    with ctx:
        # ---- pools ----
        consts = ctx.enter_context(tc.tile_pool(name="consts", bufs=1))
        x_pool = ctx.enter_context(tc.tile_pool(name="x", bufs=96))
        vt_pool = ctx.enter_context(tc.tile_pool(name="vt", bufs=2))
        pt_pool = ctx.enter_context(tc.tile_pool(name="pt", bufs=4))
        acc_pool = ctx.enter_context(tc.tile_pool(name="acc", bufs=2))
        osb_pool = ctx.enter_context(tc.tile_pool(name="osb", bufs=2))
        on_pool = ctx.enter_context(tc.tile_pool(name="on", bufs=4))
        sm_pool = ctx.enter_context(tc.tile_pool(name="sm", bufs=4))
        s_ps = ctx.enter_context(tc.tile_pool(name="sps", bufs=3, space="PSUM"))
        o_ps = ctx.enter_context(tc.tile_pool(name="ops", bufs=2, space="PSUM"))
        t_ps = ctx.enter_context(tc.tile_pool(name="tps", bufs=2, space="PSUM"))
        w_ps = ctx.enter_context(tc.tile_pool(name="wps", bufs=1, space="PSUM"))

        # ---- warmup: ramp the PE clock while DMAs are in flight ----
        wu = consts.tile([P, 512], f16)
        nc.vector.memset(wu[:], 0.0)
        wu_ps = w_ps.tile([P, 512], f32, tag="wps")
        for i in range(8):
            nc.tensor.matmul(
                wu_ps[:], lhsT=wu[:, :P], rhs=wu[:],
                start=(i == 0), stop=(i == 7),
            )

        # ---- persistent SBUF tensors ----
        identity = consts.tile([P, P], f16)
        ones32 = consts.tile([P, 1], f32)
        wq_sb = consts.tile([P, NB_E, P], f16)
        wk_sb = consts.tile([P, NB_E, P], f16)
        wv_sb = consts.tile([P, NB_E, P], f16)
        mask_sb = consts.tile([P, 8, 512], f16)
        kt_all = consts.tile([P, T], f16)
        v_all = consts.tile([P, T // P, P], f16)
        qt_all = consts.tile([P, TQ], f16)

        nc.vector.memset(ones32[:], 1.0)
        make_identity(nc, identity[:])

        def load_xq_tiles(tt):
            tiles = []
            for c in range(NB_E):
                xt = x_pool.tile([P, 512], f16, tag="x")
                nc.sync.dma_start(
                    xt[:], xqT[c * P:(c + 1) * P, tt * 512:(tt + 1) * 512])
                tiles.append(xt)
            return tiles

        def load_x_tiles(t0):
            tiles = []
            for c in range(NB_E):
                xt = x_pool.tile([P, 512], f16, tag="x")
                nc.sync.dma_start(xt[:], xT[c * P:(c + 1) * P, t0:t0 + 512])
                tiles.append(xt)
            return tiles

        def project(w_sb, x_tiles, dst_ap):
            ps = s_ps.tile([P, 512], f32, tag="sps")
            for c in range(NB_E):
                nc.tensor.matmul(
                    ps[:],
                    lhsT=w_sb[:, c, :],
                    rhs=x_tiles[c][:],
                    start=(c == 0),
                    stop=(c == NB_E - 1),
                )
            nc.scalar.copy(dst_ap, ps[:])

        # ---- round-0 DMAs in latency order ----
        xq0 = load_xq_tiles(0)
        nc.sync.dma_start(wq_sb[:], wq)
        nc.sync.dma_start(wk_sb[:], wk)
        nc.sync.dma_start(wv_sb[:], wv)
        xk0 = load_x_tiles(0)
        xk1 = load_x_tiles(512)
        nc.sync.dma_start(mask_sb[:], maskT)

        # ---- rounds: interleave projections with attention q-tiles ----
        for tt in range(N_QT):
            # Q projection for q-tile tt
            xq_tiles = xq0 if tt == 0 else load_xq_tiles(tt)
            project(wq_sb, xq_tiles, qt_all[:, tt * 512:(tt + 1) * 512])

            # K/V projections for token tiles 2tt, 2tt+1
            for tok in (2 * tt, 2 * tt + 1):
                if tt == 0:
                    xk_tiles = xk0 if tok == 0 else xk1
                else:
                    xk_tiles = load_x_tiles(tok * 512)
                project(wk_sb, xk_tiles, kt_all[:, tok * 512:(tok + 1) * 512])
                vt = vt_pool.tile([P, 512], f16, tag="vt")
                project(wv_sb, xk_tiles, vt[:])
                for u in range(4):
                    kb = tok * 4 + u
                    tp = t_ps.tile([P, P], f16, tag="tps")
                    nc.tensor.transpose(tp[:], vt[:, u * P:(u + 1) * P], identity[:])
                    nc.vector.tensor_copy(v_all[:, kb, :], tp[:])

            # attention for q-tile tt
            qs = qt_all[:, tt * 512:(tt + 1) * 512]
            ot = o_ps.tile([P, 512], f32, tag="ops")
            acc = acc_pool.tile([P, 512], f32, tag="acc")
            nkb = 8 * tt + 8

            s_tiles = [None] * nkb

            def c0_of(kb):
                if kb < 8 * tt:
                    return 0
                j = kb - 8 * tt
                return P * max(0, -(-(128 * j - 255) // 256))

            def emit_scores(kb):
                c0 = c0_of(kb)
                s = s_ps.tile([P, 512], f32, tag="sps", name=f"s_{tt}_{kb}")
                nc.tensor.matmul(
                    s[:, c0:512],
                    lhsT=kt_all[:, kb * P:(kb + 1) * P],
                    rhs=qs[:, c0:512],
                    start=True,
                    stop=True,
                )
                s_tiles[kb] = s

            emit_scores(0)
            for kb in range(nkb):
                if kb + 1 < nkb:
                    emit_scores(kb + 1)
                s = s_tiles[kb]
                c0 = c0_of(kb)
                if kb >= 8 * tt:
                    # the causal boundary lives in a single 128-col block
                    j = kb - 8 * tt
                    nc.vector.tensor_add(
                        s[:, c0:c0 + P], s[:, c0:c0 + P],
                        mask_sb[:, j, c0:c0 + P])
                pt = pt_pool.tile([P, 512], f16, tag="pt")
                nc.scalar.activation(pt[:, c0:512], s[:, c0:512], EXP, scale=SCALE)
                nc.tensor.matmul(
                    ot[:, c0:512],
                    lhsT=v_all[:, kb, :],
                    rhs=pt[:, c0:512],
                    start=(kb == 0),
                    stop=(kb == nkb - 1),
                )
                if kb == 0:
                    nc.vector.tensor_copy(acc[:], pt[:])
                else:
                    nc.vector.tensor_add(acc[:, c0:512], acc[:, c0:512],
                                         pt[:, c0:512])

            # epilogue: normalize + transpose + store
            o_sb = osb_pool.tile([P, 512], f16, tag="osb")
            nc.scalar.copy(o_sb[:], ot[:])
            on = on_pool.tile([P, 4, P], f32, tag="on")
            for i in range(4):
                lc = t_ps.tile([P, 1], f32, tag="tps", name=f"lc_{tt}_{i}")
                nc.tensor.matmul(
                    lc[:],
                    lhsT=acc[:, i * P:(i + 1) * P],
                    rhs=ones32[:],
                    start=True,
                    stop=True,
                )
                rlc = sm_pool.tile([P, 1], f32, tag="rlc")
                nc.vector.reciprocal(rlc[:], lc[:])
                tp = t_ps.tile([P, P], f16, tag="tps", name=f"otp_{tt}_{i}")
                nc.tensor.transpose(tp[:], o_sb[:, i * P:(i + 1) * P], identity[:])
                nc.vector.tensor_scalar_mul(on[:, i, :], tp[:], rlc[:])
            nc.gpsimd.dma_start(
                out[tt * 512:(tt + 1) * 512, :].rearrange("(i p) h -> p i h", p=P),
                on[:],
            )


def build_program():
    import concourse.tile as tile
    from concourse import bacc, mybir

    f32 = mybir.dt.float32
    f16 = mybir.dt.float16
    nc = bacc.Bacc("TRN2", target_bir_lowering=False, debug=False,
                   num_devices=N_CORES)
    xT = nc.dram_tensor("xT", [E, T], f16, kind="ExternalInput").ap()
    xqT = nc.dram_tensor("xqT", [E, TQ], f16, kind="ExternalInput").ap()
    wq = nc.dram_tensor("wq", [P, NB_E, P], f16, kind="ExternalInput").ap()
    wk = nc.dram_tensor("wk", [P, NB_E, P], f16, kind="ExternalInput").ap()
    wv = nc.dram_tensor("wv", [P, NB_E, P], f16, kind="ExternalInput").ap()
    maskT = nc.dram_tensor("maskT", [P, 8, 512], f16, kind="ExternalInput").ap()
    out = nc.dram_tensor("out", [TQ, H], f32, kind="ExternalOutput").ap()

    with tile.TileContext(nc) as tc:
        _emit(tc, (xT, xqT, wq, wk, wv, maskT, out))
    nc.compile()
    return nc


def _weight_pch(W: np.ndarray) -> np.ndarray:
    """[H, E] f32 -> [128(p), 8(c), 128(h)] f16 (p-major contiguous)."""
    wt = np.asarray(W, dtype=F32).T.astype(np.float16)          # [E, H]
    return np.ascontiguousarray(wt.reshape(NB_E, P, H).transpose(1, 0, 2))


def make_in_maps(x, Wq, Wk, Wv):
    """Per-core input maps. x: [B,T,E] f32; W*: [H,E] f32."""
    x = np.asarray(x, dtype=F32)
    wq_t = _weight_pch(Wq)
    wk_t = _weight_pch(Wk)
    wv_t = _weight_pch(Wv)
    in_maps = []
    for c in range(N_CORES):
        b, p = c // 2, c % 2
        xb = x[b]                                              # [T, E]
        in_maps.append({
            "xT": np.ascontiguousarray(xb.T.astype(np.float16)),
            "xqT": np.ascontiguousarray(
                xb[_query_rows(p)].T.astype(np.float16)),
            "maskT": _mask_strip(p),
            "wq": wq_t,
            "wk": wk_t,
            "wv": wv_t,
        })
    return in_maps


def run(x, Wq, Wk, Wv, trace=False, trace_cores=None):
    """Returns (full_output [B,T,H] f32, BassKernelResults)."""
    from concourse.bass_utils import run_bass_kernel_spmd

    nc = build_program()
    in_maps = make_in_maps(x, Wq, Wk, Wv)
    res = run_bass_kernel_spmd(
        nc, in_maps, list(range(N_CORES)), trace=trace,
        trace_cores=trace_cores,
    )
    full = np.empty((B, T, H), dtype=F32)
    for c in range(N_CORES):
        b, p = c // 2, c % 2
        full[b, _query_rows(p), :] = res.results[c]["out"]
    return full, res


def kernel(x, Wq, Wk, Wv):
    full, _ = run(x, Wq, Wk, Wv, trace=False)
    return full


if __name__ == "__main__":
    # quick smoke: build program only
    nc = build_program()
    print("program built ok")


# revision 12
# speedup vs baseline: 1.1953x; 1.0561x over previous
"""Single-head causal attention (B=4, T=4096, E=1024, H=128) on 8 trn2 cores.

Sharding: core c -> (batch b = c//2, piece p = c%2). Within a batch the 32
query blocks of 128 rows are split even/odd between the two pieces so the
causal workload balances. The device program is identical on all cores
(SPMD); all per-core differences are carried by the input data (a causal
boundary mask strip; queries are gathered on-device from xT).

Device algorithm (per core, "transposed" layouts):
  warmup: 8 dummy matmuls at t~0 ramp the PE p-state during the DMA wait
  QT = Wq @ xq^T           [H=128, 2048]   (query cols gathered from xT)
  KT = Wk @ x^T            [H=128, 4096]
  VT = Wv @ x^T  -> PE-transpose -> V blocks [128 tok, 128 h]
  per q-tile (512 queries): for kb in 0..8tt+7:
    ST[kb] = KT_blk^T @ QT_tile   [128 k, 512 q]  (PSUM)
    diag blocks get an additive causal mask strip (f16, per-core data)
    PT = exp(scale * ST)          (ACT, PSUM->SBUF f16)
    OT += V_blk^T @ PT            [128 h, 512 q]  (PSUM accum)
    acc += PT                     (DVE, SBUF f32) -- replaces the per-kb
                                  ones^T matmul for the softmax denominator
  epilogue per 128-q block i:
    lc_i = acc_blk^T @ ones       [128 q, 1]  (tiny 1-row matmul)
    O_i  = (o_sb_i)^T * (1/lc_i)  (PE f16 transpose + DVE scalar mul)
"""

import numpy as np

B, T, E, H = 4, 4096, 1024, 128
P = 128
NB_E = E // P           # 8 contraction chunks
TQ = T // 2             # 2048 gathered queries per core
N_QT = TQ // 512        # 4 q-tiles per core
SCALE = float(H) ** -0.5
NEG = -30000.0
N_CORES = 8
F32 = np.float32


def _query_rows(p: int) -> np.ndarray:
    """Absolute row indices of the gathered queries for piece p (in order)."""
    blocks = [np.arange(256 * g + 128 * p, 256 * g + 128 * p + 128) for g in range(16)]
    return np.concatenate(blocks)


def _mask_strip(p: int) -> np.ndarray:
    """mask [128 kk, 8 j, 512 q] f16: 0 where key visible, NEG where masked.

    Key row = 128*j + kk (in-strip block j); col 128*i + r is in-tile query
    block i. Visible iff 128*j + kk <= 256*i + 128*p + r.
    """
    kk = np.arange(1024)[:, None]           # 128*j + kk
    qq = np.arange(512)[None, :]
    i, r = qq // 128, qq % 128
    visible = kk <= 256 * i + 128 * p + r
    m = np.where(visible, 0.0, NEG).astype(np.float16)   # [1024, 512]
    return np.ascontiguousarray(m.reshape(8, 128, 512).transpose(1, 0, 2))


def _emit(tc, aps, piece_of_core=None):
    import concourse.bass as bass
    from concourse import mybir
    from concourse.masks import make_identity

    nc = tc.nc
    f32 = mybir.dt.float32
    f16 = mybir.dt.float16
    EXP = mybir.ActivationFunctionType.Exp

    xT, xqT, wq, wk, wv, maskT, out = aps

    from contextlib import ExitStack

    ctx = ExitStack()
    with ctx:
        # ---- pools ----
        consts = ctx.enter_context(tc.tile_pool(name="consts", bufs=1))
        x_pool = ctx.enter_context(tc.tile_pool(name="x", bufs=12))
        vt_pool = ctx.enter_context(tc.tile_pool(name="vt", bufs=2))
        pt_pool = ctx.enter_context(tc.tile_pool(name="pt", bufs=4))
        acc_pool = ctx.enter_context(tc.tile_pool(name="acc", bufs=2))
        osb_pool = ctx.enter_context(tc.tile_pool(name="osb", bufs=4))
        on_pool = ctx.enter_context(tc.tile_pool(name="on", bufs=4))
        sm_pool = ctx.enter_context(tc.tile_pool(name="sm", bufs=4))
        s_ps = ctx.enter_context(tc.tile_pool(name="sps", bufs=3, space="PSUM"))
        o_ps = ctx.enter_context(tc.tile_pool(name="ops", bufs=2, space="PSUM"))
        t_ps = ctx.enter_context(tc.tile_pool(name="tps", bufs=2, space="PSUM"))
        w_ps = ctx.enter_context(tc.tile_pool(name="wps", bufs=1, space="PSUM"))

        # ---- warmup: ramp the PE clock while DMAs are in flight ----
        wu = consts.tile([P, 512], f16)
        nc.vector.memset(wu[:], 0.0)
        wu_ps = w_ps.tile([P, 512], f32, tag="wps")
        NWU = 12
        for i in range(NWU):
            nc.tensor.matmul(
                wu_ps[:], lhsT=wu[:, :P], rhs=wu[:],
                start=(i == 0), stop=(i == NWU - 1),
            )

        # ---- persistent SBUF tensors ----
        identity = consts.tile([P, P], f16)
        ones32 = consts.tile([P, 1], f32)
        wq_sb = consts.tile([P, NB_E, P], f16)
        wk_sb = consts.tile([P, NB_E, P], f16)
        wv_sb = consts.tile([P, NB_E, P], f16)
        mask_sb = consts.tile([P, 8, 512], f16)
        kt_all = consts.tile([P, T], f16)
        v_all = consts.tile([P, T // P, P], f16)
        qt_all = consts.tile([P, TQ], f16)

        nc.vector.memset(ones32[:], 1.0)
        make_identity(nc, identity[:])

        # chunk-major views: one dma_start loads all 8 E-chunks of a tile
        xq_cm = xqT.rearrange("(c p) q -> p c q", p=P)
        xk_cm = xT.rearrange("(c p) t -> p c t", p=P)

        def load_xq_tiles(tt):
            xt = x_pool.tile([P, NB_E, 512], f16, tag="x")
            nc.sync.dma_start(xt[:], xq_cm[:, :, tt * 512:(tt + 1) * 512])
            return xt

        def load_x_tiles(t0):
            xt = x_pool.tile([P, NB_E, 512], f16, tag="x")
            nc.sync.dma_start(xt[:], xk_cm[:, :, t0:t0 + 512])
            return xt

        def project(w_sb, x_tiles, dst_ap):
            ps = s_ps.tile([P, 512], f32, tag="sps")
            for c in range(NB_E):
                nc.tensor.matmul(
                    ps[:],
                    lhsT=w_sb[:, c, :],
                    rhs=x_tiles[:, c, :],
                    start=(c == 0),
                    stop=(c == NB_E - 1),
                )
            nc.scalar.copy(dst_ap, ps[:])

        # ---- round-0 DMAs in latency order ----
        xq0 = load_xq_tiles(0)
        nc.sync.dma_start(wq_sb[:], wq)
        nc.sync.dma_start(wk_sb[:], wk)
        nc.sync.dma_start(wv_sb[:], wv)
        xk0 = load_x_tiles(0)
        xk1 = load_x_tiles(512)
        nc.sync.dma_start(mask_sb[:], maskT)

        # ---- rounds: interleave projections with attention q-tiles ----
        for tt in range(N_QT):
            # Q projection for q-tile tt
            xq_tiles = xq0 if tt == 0 else load_xq_tiles(tt)
            project(wq_sb, xq_tiles, qt_all[:, tt * 512:(tt + 1) * 512])

            # K/V projections for token tiles 2tt, 2tt+1
            for tok in (2 * tt, 2 * tt + 1):
                if tt == 0:
                    xk_tiles = xk0 if tok == 0 else xk1
                else:
                    xk_tiles = load_x_tiles(tok * 512)
                project(wk_sb, xk_tiles, kt_all[:, tok * 512:(tok + 1) * 512])
                vt = vt_pool.tile([P, 512], f16, tag="vt")
                project(wv_sb, xk_tiles, vt[:])
                for u in range(4):
                    kb = tok * 4 + u
                    tp = t_ps.tile([P, P], f16, tag="tps")
                    nc.tensor.transpose(tp[:], vt[:, u * P:(u + 1) * P], identity[:])
                    nc.vector.tensor_copy(v_all[:, kb, :], tp[:])

            # attention for q-tile tt
            qs = qt_all[:, tt * 512:(tt + 1) * 512]
            ot = o_ps.tile([P, 512], f32, tag="ops")
            acc = acc_pool.tile([P, 512], f32, tag="acc")
            nkb = 8 * tt + 8

            s_tiles = [None] * nkb

            def c0_of(kb):
                if kb < 8 * tt:
                    return 0
                j = kb - 8 * tt
                return P * max(0, -(-(128 * j - 255) // 256))

            def emit_scores(kb):
                c0 = c0_of(kb)
                s = s_ps.tile([P, 512], f32, tag="sps", name=f"s_{tt}_{kb}")
                nc.tensor.matmul(
                    s[:, c0:512],
                    lhsT=kt_all[:, kb * P:(kb + 1) * P],
                    rhs=qs[:, c0:512],
                    start=True,
                    stop=True,
                )
                s_tiles[kb] = s

            emit_scores(0)
            for kb in range(nkb):
                if kb + 1 < nkb:
                    emit_scores(kb + 1)
                s = s_tiles[kb]
                c0 = c0_of(kb)
                if kb >= 8 * tt:
                    # the causal boundary lives in a single 128-col block
                    j = kb - 8 * tt
                    nc.vector.tensor_add(
                        s[:, c0:c0 + P], s[:, c0:c0 + P],
                        mask_sb[:, j, c0:c0 + P])
                pt = pt_pool.tile([P, 512], f16, tag="pt")
                nc.scalar.activation(pt[:, c0:512], s[:, c0:512], EXP, scale=SCALE)
                nc.tensor.matmul(
                    ot[:, c0:512],
                    lhsT=v_all[:, kb, :],
                    rhs=pt[:, c0:512],
                    start=(kb == 0),
                    stop=(kb == nkb - 1),
                )
                if kb == 0:
                    nc.vector.tensor_copy(acc[:], pt[:])
                else:
                    nc.vector.tensor_add(acc[:, c0:512], acc[:, c0:512],
                                         pt[:, c0:512])

            # epilogue: per-128-block normalize + transpose + store pipeline
            on = on_pool.tile([P, 4, P], f32, tag="on")
            for i in range(4):
                lc = t_ps.tile([P, 1], f32, tag="tps", name=f"lc_{tt}_{i}")
                nc.tensor.matmul(
                    lc[:],
                    lhsT=acc[:, i * P:(i + 1) * P],
                    rhs=ones32[:],
                    start=True,
                    stop=True,
                )
                rlc = sm_pool.tile([P, 1], f32, tag="rlc")
                nc.vector.reciprocal(rlc[:], lc[:])
                o_sb = osb_pool.tile([P, P], f16, tag="osb")
                nc.vector.tensor_copy(o_sb[:], ot[:, i * P:(i + 1) * P])
                tp = t_ps.tile([P, P], f16, tag="tps", name=f"otp_{tt}_{i}")
                nc.tensor.transpose(tp[:], o_sb[:], identity[:])
                nc.vector.tensor_scalar_mul(on[:, i, :], tp[:], rlc[:])
                r0 = tt * 512 + i * P
                nc.gpsimd.dma_start(out[r0:r0 + P, :], on[:, i, :])


def build_program():
    import concourse.tile as tile
    from concourse import bacc, mybir

    f32 = mybir.dt.float32
    f16 = mybir.dt.float16
    nc = bacc.Bacc("TRN2", target_bir_lowering=False, debug=False,
                   num_devices=N_CORES)
    xT = nc.dram_tensor("xT", [E, T], f16, kind="ExternalInput").ap()
    xqT = nc.dram_tensor("xqT", [E, TQ], f16, kind="ExternalInput").ap()
    wq = nc.dram_tensor("wq", [P, NB_E, P], f16, kind="ExternalInput").ap()
    wk = nc.dram_tensor("wk", [P, NB_E, P], f16, kind="ExternalInput").ap()
    wv = nc.dram_tensor("wv", [P, NB_E, P], f16, kind="ExternalInput").ap()
    maskT = nc.dram_tensor("maskT", [P, 8, 512], f16, kind="ExternalInput").ap()
    out = nc.dram_tensor("out", [TQ, H], f32, kind="ExternalOutput").ap()

    with tile.TileContext(nc) as tc:
        _emit(tc, (xT, xqT, wq, wk, wv, maskT, out))
    nc.compile()
    return nc


def _weight_pch(W: np.ndarray) -> np.ndarray:
    """[H, E] f32 -> [128(p), 8(c), 128(h)] f16 (p-major contiguous)."""
    wt = np.asarray(W, dtype=F32).T.astype(np.float16)          # [E, H]
    return np.ascontiguousarray(wt.reshape(NB_E, P, H).transpose(1, 0, 2))


def make_in_maps(x, Wq, Wk, Wv):
    """Per-core input maps. x: [B,T,E] f32; W*: [H,E] f32."""
    x = np.asarray(x, dtype=F32)
    wq_t = _weight_pch(Wq)
    wk_t = _weight_pch(Wk)
    wv_t = _weight_pch(Wv)
    in_maps = []
    for c in range(N_CORES):
        b, p = c // 2, c % 2
        xb = x[b]                                              # [T, E]
        in_maps.append({
            "xT": np.ascontiguousarray(xb.T.astype(np.float16)),
            "xqT": np.ascontiguousarray(
                xb[_query_rows(p)].T.astype(np.float16)),
            "maskT": _mask_strip(p),
            "wq": wq_t,
            "wk": wk_t,
            "wv": wv_t,
        })
    return in_maps


def run(x, Wq, Wk, Wv, trace=False, trace_cores=None):
    """Returns (full_output [B,T,H] f32, BassKernelResults)."""
    from concourse.bass_utils import run_bass_kernel_spmd

    nc = build_program()
    in_maps = make_in_maps(x, Wq, Wk, Wv)
    res = run_bass_kernel_spmd(
        nc, in_maps, list(range(N_CORES)), trace=trace,
        trace_cores=trace_cores,
    )
    full = np.empty((B, T, H), dtype=F32)
    for c in range(N_CORES):
        b, p = c // 2, c % 2
        full[b, _query_rows(p), :] = res.results[c]["out"]
    return full, res


def kernel(x, Wq, Wk, Wv):
    full, _ = run(x, Wq, Wk, Wv, trace=False)
    return full


if __name__ == "__main__":
    # quick smoke: build program only
    nc = build_program()
    print("program built ok")


# revision 15
# speedup vs baseline: 1.3474x; 1.1272x over previous
"""Single-head causal attention (B=4, T=4096, E=1024, H=128) on 8 trn2 cores.

Sharding: core c -> (batch b = c//2, piece p = c%2). Within a batch the 32
query blocks of 128 rows are split even/odd between the two pieces so the
causal workload balances. The device program is identical on all cores
(SPMD); per-core differences are carried by the input data (gathered query
columns xqT + a 0/1 causal-boundary mask strip).

Device algorithm (per core, "transposed" layouts):
  warmup: dummy matmuls at t~0 ramp the PE p-state during the DMA wait
  QT = Wq @ xq^T           [H=128, 2048]
  KT = Wk @ x^T            [H=128, 4096]
  VT = Wv @ x^T  -> PE-transpose -> V blocks [128 tok, 128 h]
  per q-tile (512 queries): for kb:
    ST[kb] = KT_blk^T @ QT_tile   [128 k, 512 q]  (PSUM)
    PT = exp(scale * ST)          (ACT, PSUM->SBUF f16)
    diag blocks: PT *= tri mask   (DVE f16, multiplicative - keeps ACT free)
    OT += V_blk^T @ PT            (PSUM accum)
    acc += PT                     (DVE SBUF f32 - softmax denominator)
  The NEXT round's projection matmuls are interleaved into the attention
  k-block stream so the PE never idles while ACT computes exp (idle PE
  triggers DVFS down-clocking that halves matmul throughput).
  epilogue per 128-q block i:
    lc_i = acc_blk^T @ ones; O_i = (OT_i)^T * (1/lc_i); 128-row store DMA
"""

import numpy as np

B, T, E, H = 4, 4096, 1024, 128
P = 128
NB_E = E // P           # 8 contraction chunks
TQ = T // 2             # 2048 gathered queries per core
N_QT = TQ // 512        # 4 q-tiles per core
SCALE = float(H) ** -0.5
N_CORES = 8
F32 = np.float32


def _query_rows(p: int) -> np.ndarray:
    """Absolute row indices of the gathered queries for piece p (in order)."""
    blocks = [np.arange(256 * g + 128 * p, 256 * g + 128 * p + 128) for g in range(16)]
    return np.concatenate(blocks)


def _mask_strip(p: int) -> np.ndarray:
    """mask [128 kk, 8 j, 512 q] f16: 1 where key visible, 0 where masked."""
    kk = np.arange(1024)[:, None]           # 128*j + kk
    qq = np.arange(512)[None, :]
    i, r = qq // 128, qq % 128
    visible = kk <= 256 * i + 128 * p + r
    m = visible.astype(np.float16)          # [1024, 512]
    return np.ascontiguousarray(m.reshape(8, 128, 512).transpose(1, 0, 2))


def _emit(tc, aps):
    import concourse.bass as bass
    from concourse import mybir
    from concourse.masks import make_identity

    nc = tc.nc
    f32 = mybir.dt.float32
    f16 = mybir.dt.float16
    EXP = mybir.ActivationFunctionType.Exp

    xT, xqT, wq, wk, wv, maskT, out = aps

    from contextlib import ExitStack

    ctx = ExitStack()
    with ctx:
        # ---- pools ----
        consts = ctx.enter_context(tc.tile_pool(name="consts", bufs=1))
        x_pool = ctx.enter_context(tc.tile_pool(name="x", bufs=12))
        vt_pool = ctx.enter_context(tc.tile_pool(name="vt", bufs=2))
        pt_pool = ctx.enter_context(tc.tile_pool(name="pt", bufs=4))
        acc_pool = ctx.enter_context(tc.tile_pool(name="acc", bufs=2))
        osb_pool = ctx.enter_context(tc.tile_pool(name="osb", bufs=4))
        on_pool = ctx.enter_context(tc.tile_pool(name="on", bufs=4))
        sm_pool = ctx.enter_context(tc.tile_pool(name="sm", bufs=4))
        s_ps = ctx.enter_context(tc.tile_pool(name="sps", bufs=2, space="PSUM"))
        p_ps = ctx.enter_context(tc.tile_pool(name="pps", bufs=2, space="PSUM"))
        o_ps = ctx.enter_context(tc.tile_pool(name="ops", bufs=2, space="PSUM"))
        t_ps = ctx.enter_context(tc.tile_pool(name="tps", bufs=2, space="PSUM"))

        # ---- warmup: ramp the PE clock while DMAs are in flight ----
        wu = consts.tile([P, 512], f16)
        nc.vector.memset(wu[:], 0.0)
        wu_ps = t_ps.tile([P, 512], f32, tag="tps", name="warm")
        NWU = 12
        for i in range(NWU):
            nc.tensor.matmul(
                wu_ps[:], lhsT=wu[:, :P], rhs=wu[:],
                start=(i == 0), stop=(i == NWU - 1),
            )

        # ---- persistent SBUF tensors ----
        identity = consts.tile([P, P], f16)
        ones32 = consts.tile([P, 1], f32)
        wq_sb = consts.tile([P, NB_E, P], f16)
        wk_sb = consts.tile([P, NB_E, P], f16)
        wv_sb = consts.tile([P, NB_E, P], f16)
        mask_sb = consts.tile([P, 8, 512], f16)
        kt_all = consts.tile([P, T], f16)
        v_all = consts.tile([P, T // P, P], f16)
        qt_all = consts.tile([P, TQ], f16)

        nc.vector.memset(ones32[:], 1.0)
        make_identity(nc, identity[:])

        # chunk-major views: one dma_start loads all 8 E-chunks of a tile
        xq_cm = xqT.rearrange("(c p) q -> p c q", p=P)
        xk_cm = xT.rearrange("(c p) t -> p c t", p=P)

        def load_xq_tile(tt):
            xt = x_pool.tile([P, NB_E, 512], f16, tag="x")
            nc.sync.dma_start(xt[:], xq_cm[:, :, tt * 512:(tt + 1) * 512])
            return xt

        def load_x_tile(t0):
            xt = x_pool.tile([P, NB_E, 512], f16, tag="x")
            nc.sync.dma_start(xt[:], xk_cm[:, :, t0:t0 + 512])
            return xt

        # ---- round-0 DMAs in latency order; later rounds prefetch ----
        xq_t = [None] * N_QT
        xk_t = [None] * (2 * N_QT)
        xq_t[0] = load_xq_tile(0)
        xk_t[0] = load_x_tile(0)
        nc.sync.dma_start(wq_sb[:], wq)
        nc.sync.dma_start(wk_sb[:], wk)
        xk_t[1] = load_x_tile(512)
        nc.sync.dma_start(wv_sb[:], wv)
        nc.sync.dma_start(mask_sb[:], maskT)
        for tt in range(1, N_QT):
            xq_t[tt] = load_xq_tile(tt)
            xk_t[2 * tt] = load_x_tile(2 * tt * 512)
            xk_t[2 * tt + 1] = load_x_tile((2 * tt + 1) * 512)

        # ---- projection work-item machinery ----
        # Each round's projections are emitted as a list of small closures
        # (one PE op each); the attention loop interleaves them between
        # score/PV matmuls so the PE pipeline never drains.
        _uid = [0]

        def proj_items(w_sb, x_tile, dst_ap, vt_tok=None):
            """Items for one projection [128, 512]. If vt_tok is not None,
            the result is V: transpose blocks into v_all instead of copy."""
            state = {}
            _uid[0] += 1
            uid = _uid[0]

            def mk_mm(c):
                def it():
                    if c == 0:
                        state["ps"] = p_ps.tile([P, 512], f32, tag="pps",
                                                name=f"pps_{uid}")
                    nc.tensor.matmul(
                        state["ps"][:],
                        lhsT=w_sb[:, c, :],
                        rhs=x_tile[:, c, :],
                        start=(c == 0),
                        stop=(c == NB_E - 1),
                    )
                    if c == NB_E - 1 and vt_tok is None:
                        nc.scalar.copy(dst_ap, state["ps"][:])
                return it

            items = [mk_mm(c) for c in range(NB_E)]
            if vt_tok is not None:
                def cp():
                    vt = vt_pool.tile([P, 512], f16, tag="vt",
                                      name=f"vt_{uid}")
                    state["vt"] = vt
                    nc.scalar.copy(vt[:], state["ps"][:])
                items.append(cp)

                def mk_tr(u):
                    def it():
                        kb = vt_tok * 4 + u
                        tp = t_ps.tile([P, P], f16, tag="tps",
                                       name=f"vtr_{kb}")
                        nc.tensor.transpose(
                            tp[:], state["vt"][:, u * P:(u + 1) * P],
                            identity[:])
                        nc.vector.tensor_copy(v_all[:, kb, :], tp[:])
                    return it
                items += [mk_tr(u) for u in range(4)]
            return items

        def round_proj_items(tt):
            items = []
            items += proj_items(wq_sb, xq_t[tt],
                                qt_all[:, tt * 512:(tt + 1) * 512])
            for tok in (2 * tt, 2 * tt + 1):
                items += proj_items(
                    wk_sb, xk_t[tok],
                    kt_all[:, tok * 512:(tok + 1) * 512])
                items += proj_items(wv_sb, xk_t[tok], None, vt_tok=tok)
            return items

        # ---- round 0 projections run straight (DMA-paced prologue) ----
        for it in round_proj_items(0):
            it()

        # ---- attention rounds, fused with next round's projections ----
        for tt in range(N_QT):
            work = round_proj_items(tt + 1) if tt + 1 < N_QT else []
            qs = qt_all[:, tt * 512:(tt + 1) * 512]
            ot = o_ps.tile([P, 512], f32, tag="ops")
            acc = acc_pool.tile([P, 512], f32, tag="acc")
            nkb = 8 * tt + 8

            s_tiles = [None] * nkb

            def c0_of(kb):
                if kb < 8 * tt:
                    return 0
                j = kb - 8 * tt
                return P * max(0, -(-(128 * j - 255) // 256))

            def emit_scores(kb):
                c0 = c0_of(kb)
                s = s_ps.tile([P, 512], f32, tag="sps", name=f"s_{tt}_{kb}")
                nc.tensor.matmul(
                    s[:, c0:512],
                    lhsT=kt_all[:, kb * P:(kb + 1) * P],
                    rhs=qs[:, c0:512],
                    start=True,
                    stop=True,
                )
                s_tiles[kb] = s

            emit_scores(0)
            wi = 0  # work items emitted
            for kb in range(nkb):
                if kb + 1 < nkb:
                    emit_scores(kb + 1)
                # interleave a fair share of next-round projection work
                quota = ((kb + 1) * len(work) + nkb - 1) // nkb
                while wi < quota:
                    work[wi]()
                    wi += 1
                s = s_tiles[kb]
                c0 = c0_of(kb)
                pt = pt_pool.tile([P, 512], f16, tag="pt")
                nc.scalar.activation(pt[:, c0:512], s[:, c0:512], EXP,
                                     scale=SCALE)
                if kb >= 8 * tt:
                    # zero the masked part of the boundary 128-col block
                    j = kb - 8 * tt
                    nc.vector.tensor_mul(
                        pt[:, c0:c0 + P], pt[:, c0:c0 + P],
                        mask_sb[:, j, c0:c0 + P])
                nc.tensor.matmul(
                    ot[:, c0:512],
                    lhsT=v_all[:, kb, :],
                    rhs=pt[:, c0:512],
                    start=(kb == 0),
                    stop=(kb == nkb - 1),
                )
                if kb == 0:
                    nc.vector.tensor_copy(acc[:], pt[:])
                else:
                    nc.vector.tensor_add(acc[:, c0:512], acc[:, c0:512],
                                         pt[:, c0:512])
            assert wi == len(work)

            # epilogue: per-128-block normalize + transpose + store pipeline
            on = on_pool.tile([P, 4, P], f32, tag="on")
            for i in range(4):
                lc = t_ps.tile([P, 1], f32, tag="tps", name=f"lc_{tt}_{i}")
                nc.tensor.matmul(
                    lc[:],
                    lhsT=acc[:, i * P:(i + 1) * P],
                    rhs=ones32[:],
                    start=True,
                    stop=True,
                )
                rlc = sm_pool.tile([P, 1], f32, tag="rlc")
                nc.vector.reciprocal(rlc[:], lc[:])
                o_sb = osb_pool.tile([P, P], f16, tag="osb")
                nc.vector.tensor_copy(o_sb[:], ot[:, i * P:(i + 1) * P])
                tp = t_ps.tile([P, P], f16, tag="tps", name=f"otp_{tt}_{i}")
                nc.tensor.transpose(tp[:], o_sb[:], identity[:])
                nc.vector.tensor_scalar_mul(on[:, i, :], tp[:], rlc[:])
                r0 = tt * 512 + i * P
                nc.gpsimd.dma_start(out[r0:r0 + P, :], on[:, i, :])


def build_program():
    import concourse.tile as tile
    from concourse import bacc, mybir

    f32 = mybir.dt.float32
    f16 = mybir.dt.float16
    nc = bacc.Bacc("TRN2", target_bir_lowering=False, debug=False,
                   num_devices=N_CORES)
    xT = nc.dram_tensor("xT", [E, T], f16, kind="ExternalInput").ap()
    xqT = nc.dram_tensor("xqT", [E, TQ], f16, kind="ExternalInput").ap()
    wq = nc.dram_tensor("wq", [P, NB_E, P], f16, kind="ExternalInput").ap()
    wk = nc.dram_tensor("wk", [P, NB_E, P], f16, kind="ExternalInput").ap()
    wv = nc.dram_tensor("wv", [P, NB_E, P], f16, kind="ExternalInput").ap()
    maskT = nc.dram_tensor("maskT", [P, 8, 512], f16, kind="ExternalInput").ap()
    out = nc.dram_tensor("out", [TQ, H], f32, kind="ExternalOutput").ap()

    with tile.TileContext(nc) as tc:
        _emit(tc, (xT, xqT, wq, wk, wv, maskT, out))
    nc.compile()
    return nc


def _weight_pch(W: np.ndarray) -> np.ndarray:
    """[H, E] f32 -> [128(p), 8(c), 128(h)] f16 (p-major contiguous)."""
    wt = np.asarray(W, dtype=F32).T.astype(np.float16)          # [E, H]
    return np.ascontiguousarray(wt.reshape(NB_E, P, H).transpose(1, 0, 2))


def make_in_maps(x, Wq, Wk, Wv):
    """Per-core input maps. x: [B,T,E] f32; W*: [H,E] f32."""
    x = np.asarray(x, dtype=F32)
    wq_t = _weight_pch(Wq)
    wk_t = _weight_pch(Wk)
    wv_t = _weight_pch(Wv)
    in_maps = []
    for c in range(N_CORES):
        b, p = c // 2, c % 2
        xb = x[b]                                              # [T, E]
        in_maps.append({
            "xT": np.ascontiguousarray(xb.T.astype(np.float16)),
            "xqT": np.ascontiguousarray(
                xb[_query_rows(p)].T.astype(np.float16)),
            "maskT": _mask_strip(p),
            "wq": wq_t,
            "wk": wk_t,
            "wv": wv_t,
        })
    return in_maps


def run(x, Wq, Wk, Wv, trace=False, trace_cores=None):
    """Returns (full_output [B,T,H] f32, BassKernelResults)."""
    from concourse.bass_utils import run_bass_kernel_spmd

    nc = build_program()
    in_maps = make_in_maps(x, Wq, Wk, Wv)
    res = run_bass_kernel_spmd(
        nc, in_maps, list(range(N_CORES)), trace=trace,
        trace_cores=trace_cores,
    )
    full = np.empty((B, T, H), dtype=F32)
    for c in range(N_CORES):
        b, p = c // 2, c % 2
        full[b, _query_rows(p), :] = res.results[c]["out"]
    return full, res


def kernel(x, Wq, Wk, Wv):
    full, _ = run(x, Wq, Wk, Wv, trace=False)
    return full


if __name__ == "__main__":
    # quick smoke: build program only
    nc = build_program()
    print("program built ok")


# revision 21
# speedup vs baseline: 1.3997x; 1.0388x over previous
"""Single-head causal attention (B=4, T=4096, E=1024, H=128) on 8 trn2 cores.

Sharding: core c -> (batch b = c//2, piece p = c%2). Within a batch the 32
query blocks of 128 rows are split even/odd between the two pieces so the
causal workload balances. The device program is identical on all cores
(SPMD); per-core differences are carried by the input data (gathered query
columns xqT + a 0/1 causal-boundary mask strip).

Device algorithm (per core, "transposed" layouts):
  warmup: dummy matmuls at t~0 ramp the PE p-state during the DMA wait
  QT = Wq @ xq^T           [H=128, 2048]
  KT = Wk @ x^T            [H=128, 4096]
  VT = Wv @ x^T  -> PE-transpose -> V blocks [128 tok, 128 h]
  per q-tile (512 queries): for kb:
    ST[kb] = KT_blk^T @ QT_tile   [128 k, 512 q]  (PSUM)
    PT = exp(scale * ST)          (ACT, PSUM->SBUF f16)
    diag blocks: PT *= tri mask   (DVE f16, multiplicative - keeps ACT free)
    OT += V_blk^T @ PT            (PSUM accum)
    acc += PT                     (DVE SBUF f32 - softmax denominator)
  The NEXT round's projection matmuls are interleaved into the attention
  k-block stream so the PE never idles while ACT computes exp (idle PE
  triggers DVFS down-clocking that halves matmul throughput).
  epilogue per 128-q block i:
    lc_i = acc_blk^T @ ones; O_i = (OT_i)^T * (1/lc_i); 128-row store DMA
"""

import numpy as np

B, T, E, H = 4, 4096, 1024, 128
P = 128
NB_E = E // P           # 8 contraction chunks
TQ = T // 2             # 2048 gathered queries per core
N_QT = TQ // 512        # 4 q-tiles per core
SCALE = float(H) ** -0.5
N_CORES = 8
F32 = np.float32


def _query_rows(p: int) -> np.ndarray:
    """Absolute row indices of the gathered queries for piece p (in order)."""
    blocks = [np.arange(256 * g + 128 * p, 256 * g + 128 * p + 128) for g in range(16)]
    return np.concatenate(blocks)


def _mask_strip(p: int) -> np.ndarray:
    """mask [128 kk, 8 j, 512 q] f16: 1 where key visible, 0 where masked."""
    kk = np.arange(1024)[:, None]           # 128*j + kk
    qq = np.arange(512)[None, :]
    i, r = qq // 128, qq % 128
    visible = kk <= 256 * i + 128 * p + r
    m = visible.astype(np.float16)          # [1024, 512]
    return np.ascontiguousarray(m.reshape(8, 128, 512).transpose(1, 0, 2))


def _emit(tc, aps):
    import concourse.bass as bass
    from concourse import mybir
    from concourse.masks import make_identity

    nc = tc.nc
    f32 = mybir.dt.float32
    f16 = mybir.dt.float16
    EXP = mybir.ActivationFunctionType.Exp

    xT, xqT, wq, wk, wv, maskT, out = aps

    from contextlib import ExitStack

    ctx = ExitStack()
    with ctx:
        # ---- pools ----
        consts = ctx.enter_context(tc.tile_pool(name="consts", bufs=1))
        x_pool = ctx.enter_context(tc.tile_pool(name="x", bufs=12))
        vt_pool = ctx.enter_context(tc.tile_pool(name="vt", bufs=2))
        pt_pool = ctx.enter_context(tc.tile_pool(name="pt", bufs=4))
        acc_pool = ctx.enter_context(tc.tile_pool(name="acc", bufs=2))
        osb_pool = ctx.enter_context(tc.tile_pool(name="osb", bufs=4))
        on_pool = ctx.enter_context(tc.tile_pool(name="on", bufs=4))
        sm_pool = ctx.enter_context(tc.tile_pool(name="sm", bufs=4))
        s_ps = ctx.enter_context(tc.tile_pool(name="sps", bufs=2, space="PSUM"))
        p_ps = ctx.enter_context(tc.tile_pool(name="pps", bufs=2, space="PSUM"))
        o_ps = ctx.enter_context(tc.tile_pool(name="ops", bufs=2, space="PSUM"))
        t_ps = ctx.enter_context(tc.tile_pool(name="tps", bufs=2, space="PSUM"))

        # ---- warmup: ramp the PE clock while DMAs are in flight ----
        wu = consts.tile([P, 512], f16)
        nc.vector.memset(wu[:], 0.0)
        wu_ps = t_ps.tile([P, 512], f32, tag="tps", name="warm")
        NWU = 13
        for i in range(NWU):
            nc.tensor.matmul(
                wu_ps[:], lhsT=wu[:, :P], rhs=wu[:],
                start=(i == 0), stop=(i == NWU - 1),
            )

        # ---- persistent SBUF tensors ----
        identity = consts.tile([P, P], f16)
        ones32 = consts.tile([P, 1], f32)
        wq_sb = consts.tile([P, NB_E, P], f16)
        wk_sb = consts.tile([P, NB_E, P], f16)
        wv_sb = consts.tile([P, NB_E, P], f16)
        mask_sb = consts.tile([P, 8, 512], f16)
        kt_all = consts.tile([P, T], f16)
        v_all = consts.tile([P, T // P, P], f16)
        qt_all = consts.tile([P, TQ], f16)

        nc.vector.memset(ones32[:], 1.0)
        make_identity(nc, identity[:])

        # chunk-major views: one dma_start loads all 8 E-chunks of a tile
        xq_cm = xqT.rearrange("(c p) q -> p c q", p=P)
        xk_cm = xT.rearrange("(c p) t -> p c t", p=P)

        class XTile:
            """x tile split into DMA parts; chunk(c) -> [128, 512] AP."""
            def __init__(self, parts, cpp):
                self.parts, self.cpp = parts, cpp

            def chunk(self, c):
                return self.parts[c // self.cpp][:, c % self.cpp, :]

        def load_tile(cm, t0, halves, nm):
            if halves:
                parts = []
                for h in range(2):
                    xt = x_pool.tile([P, 4, 512], f16, tag="x",
                                     name=f"{nm}_{h}")
                    nc.sync.dma_start(
                        xt[:], cm[:, 4 * h:4 * h + 4, t0:t0 + 512])
                    parts.append(xt)
                return XTile(parts, 4)
            xt = x_pool.tile([P, NB_E, 512], f16, tag="x", name=nm)
            nc.sync.dma_start(xt[:], cm[:, :, t0:t0 + 512])
            return XTile([xt], NB_E)

        # ---- round-0 DMAs in latency order; later rounds prefetch ----
        xq_t = [None] * N_QT
        xk_t = [None] * (2 * N_QT)
        xq_t[0] = load_tile(xq_cm, 0, True, "xq0")
        nc.sync.dma_start(wq_sb[:], wq)
        xk_t[0] = load_tile(xk_cm, 0, True, "xk0")
        nc.sync.dma_start(wk_sb[:], wk)
        xk_t[1] = load_tile(xk_cm, 512, True, "xk1")
        nc.sync.dma_start(wv_sb[:], wv)
        nc.sync.dma_start(mask_sb[:], maskT)
        for tt in range(1, N_QT):
            xq_t[tt] = load_tile(xq_cm, tt * 512, False, f"xq{tt}")
            xk_t[2 * tt] = load_tile(xk_cm, 2 * tt * 512, False, f"xk{2*tt}")
            xk_t[2 * tt + 1] = load_tile(
                xk_cm, (2 * tt + 1) * 512, False, f"xk{2*tt+1}")

        # ---- projection work-item machinery ----
        # Each round's projections are emitted as a list of small closures
        # (one PE op each); the attention loop interleaves them between
        # score/PV matmuls so the PE pipeline never drains.
        _uid = [0]

        def proj_items(w_sb, x_tile, dst_ap, vt_tok=None):
            """Items for one projection [128, 512]. If vt_tok is not None,
            the result is V: transpose blocks into v_all instead of copy."""
            state = {}
            _uid[0] += 1
            uid = _uid[0]

            def mk_mm(c):
                def it():
                    if c == 0:
                        state["ps"] = p_ps.tile([P, 512], f32, tag="pps",
                                                name=f"pps_{uid}")
                    nc.tensor.matmul(
                        state["ps"][:],
                        lhsT=w_sb[:, c, :],
                        rhs=x_tile.chunk(c),
                        start=(c == 0),
                        stop=(c == NB_E - 1),
                    )
                    if c == NB_E - 1 and vt_tok is None:
                        nc.scalar.copy(dst_ap, state["ps"][:])
                return it

            items = [mk_mm(c) for c in range(NB_E)]
            if vt_tok is not None:
                def cp():
                    vt = vt_pool.tile([P, 512], f16, tag="vt",
                                      name=f"vt_{uid}")
                    state["vt"] = vt
                    nc.scalar.copy(vt[:], state["ps"][:])
                items.append(cp)

                def mk_tr(u):
                    def it():
                        kb = vt_tok * 4 + u
                        tp = t_ps.tile([P, P], f16, tag="tps",
                                       name=f"vtr_{kb}")
                        nc.tensor.transpose(
                            tp[:], state["vt"][:, u * P:(u + 1) * P],
                            identity[:])
                        nc.vector.tensor_copy(v_all[:, kb, :], tp[:])
                    return it
                items += [mk_tr(u) for u in range(4)]
            return items

        def g_q(tt):
            return proj_items(wq_sb, xq_t[tt],
                              qt_all[:, tt * 512:(tt + 1) * 512])

        def g_kv(tok):
            items = proj_items(wk_sb, xk_t[tok],
                               kt_all[:, tok * 512:(tok + 1) * 512])
            items += proj_items(wv_sb, xk_t[tok], None, vt_tok=tok)
            return items

        def dl(items, deadline):
            return [(deadline, it) for it in items]

        # ---- round 0 prologue: Q0 + K/V tok0 run straight (DMA-paced) ----
        for it in g_q(0) + g_kv(0):
            it()

        # Work schedule: attention tt interleaves (deadline slot, item):
        #  - its own second token tile's K/V (needed from kb = 8tt+4)
        #  - projections needed before round tt+1 starts
        work_of = {
            0: dl(g_kv(1), 2) + dl(g_q(1) + g_kv(2), 7),
            1: dl(g_kv(3), 10) + dl(g_q(2) + g_kv(4), 15),
            2: dl(g_kv(5), 18) + dl(g_q(3), 23),
            3: dl(g_kv(6), 21) + dl(g_kv(7), 25),
        }

        # ---- attention rounds, fused with interleaved projections ----
        for tt in range(N_QT):
            work = work_of[tt]
            qs = qt_all[:, tt * 512:(tt + 1) * 512]
            ot = o_ps.tile([P, 512], f32, tag="ops")
            acc = acc_pool.tile([P, 512], f32, tag="acc")
            nkb = 8 * tt + 8

            s_tiles = [None] * nkb

            def c0_of(kb):
                if kb < 8 * tt:
                    return 0
                j = kb - 8 * tt
                return P * max(0, -(-(128 * j - 255) // 256))

            def emit_scores(kb):
                c0 = c0_of(kb)
                s = s_ps.tile([P, 512], f32, tag="sps", name=f"s_{tt}_{kb}")
                nc.tensor.matmul(
                    s[:, c0:512],
                    lhsT=kt_all[:, kb * P:(kb + 1) * P],
                    rhs=qs[:, c0:512],
                    start=True,
                    stop=True,
                )
                s_tiles[kb] = s

            emit_scores(0)
            wi = 0  # work items emitted
            for kb in range(nkb):
                if kb + 1 < nkb:
                    emit_scores(kb + 1)
                # interleave a fair share of the projection work; deadline
                # items are forced out regardless of the even-spread quota
                while wi < len(work) and (
                        work[wi][0] <= kb
                        or wi * nkb < (kb + 1) * len(work)):
                    work[wi][1]()
                    wi += 1
                s = s_tiles[kb]
                c0 = c0_of(kb)
                pt = pt_pool.tile([P, 512], f16, tag="pt")
                nc.scalar.activation(pt[:, c0:512], s[:, c0:512], EXP,
                                     scale=SCALE)
                if kb >= 8 * tt:
                    # zero the masked part of the boundary 128-col block
                    j = kb - 8 * tt
                    nc.vector.tensor_mul(
                        pt[:, c0:c0 + P], pt[:, c0:c0 + P],
                        mask_sb[:, j, c0:c0 + P])
                nc.tensor.matmul(
                    ot[:, c0:512],
                    lhsT=v_all[:, kb, :],
                    rhs=pt[:, c0:512],
                    start=(kb == 0),
                    stop=(kb == nkb - 1),
                )
                if kb == 0:
                    nc.vector.tensor_copy(acc[:], pt[:])
                else:
                    nc.vector.tensor_add(acc[:, c0:512], acc[:, c0:512],
                                         pt[:, c0:512])
            assert wi == len(work)

            # epilogue: per-128-block normalize + transpose + store pipeline
            on = on_pool.tile([P, 4, P], f32, tag="on")
            for i in range(4):
                lc = t_ps.tile([P, 1], f32, tag="tps", name=f"lc_{tt}_{i}")
                nc.tensor.matmul(
                    lc[:],
                    lhsT=acc[:, i * P:(i + 1) * P],
                    rhs=ones32[:],
                    start=True,
                    stop=True,
                )
                rlc = sm_pool.tile([P, 1], f32, tag="rlc")
                nc.vector.reciprocal(rlc[:], lc[:])
                o_sb = osb_pool.tile([P, P], f16, tag="osb")
                nc.vector.tensor_copy(o_sb[:], ot[:, i * P:(i + 1) * P])
                tp = t_ps.tile([P, P], f16, tag="tps", name=f"otp_{tt}_{i}")
                nc.tensor.transpose(tp[:], o_sb[:], identity[:])
                nc.vector.tensor_scalar_mul(on[:, i, :], tp[:], rlc[:])
                r0 = tt * 512 + i * P
                nc.sync.dma_start(out[r0:r0 + P, :], on[:, i, :])


def build_program():
    import concourse.tile as tile
    from concourse import bacc, mybir

    f32 = mybir.dt.float32
    f16 = mybir.dt.float16
    nc = bacc.Bacc("TRN2", target_bir_lowering=False, debug=False,
                   num_devices=N_CORES)
    xT = nc.dram_tensor("xT", [E, T], f16, kind="ExternalInput").ap()
    xqT = nc.dram_tensor("xqT", [E, TQ], f16, kind="ExternalInput").ap()
    wq = nc.dram_tensor("wq", [P, NB_E, P], f16, kind="ExternalInput").ap()
    wk = nc.dram_tensor("wk", [P, NB_E, P], f16, kind="ExternalInput").ap()
    wv = nc.dram_tensor("wv", [P, NB_E, P], f16, kind="ExternalInput").ap()
    maskT = nc.dram_tensor("maskT", [P, 8, 512], f16, kind="ExternalInput").ap()
    out = nc.dram_tensor("out", [TQ, H], f32, kind="ExternalOutput").ap()

    with tile.TileContext(nc) as tc:
        _emit(tc, (xT, xqT, wq, wk, wv, maskT, out))
    nc.compile()
    return nc


def _weight_pch(W: np.ndarray) -> np.ndarray:
    """[H, E] f32 -> [128(p), 8(c), 128(h)] f16 (p-major contiguous)."""
    wt = np.asarray(W, dtype=F32).T.astype(np.float16)          # [E, H]
    return np.ascontiguousarray(wt.reshape(NB_E, P, H).transpose(1, 0, 2))


def make_in_maps(x, Wq, Wk, Wv):
    """Per-core input maps. x: [B,T,E] f32; W*: [H,E] f32."""
    x = np.asarray(x, dtype=F32)
    wq_t = _weight_pch(Wq)
    wk_t = _weight_pch(Wk)
    wv_t = _weight_pch(Wv)
    in_maps = []
    for c in range(N_CORES):
        b, p = c // 2, c % 2
        xb = x[b]                                              # [T, E]
        in_maps.append({
            "xT": np.ascontiguousarray(xb.T.astype(np.float16)),
            "xqT": np.ascontiguousarray(
                xb[_query_rows(p)].T.astype(np.float16)),
            "maskT": _mask_strip(p),
            "wq": wq_t,
            "wk": wk_t,
            "wv": wv_t,
        })
    return in_maps


def run(x, Wq, Wk, Wv, trace=False, trace_cores=None):
    """Returns (full_output [B,T,H] f32, BassKernelResults)."""
    from concourse.bass_utils import run_bass_kernel_spmd

    nc = build_program()
    in_maps = make_in_maps(x, Wq, Wk, Wv)
    res = run_bass_kernel_spmd(
        nc, in_maps, list(range(N_CORES)), trace=trace,
        trace_cores=trace_cores,
    )
    full = np.empty((B, T, H), dtype=F32)
    for c in range(N_CORES):
        b, p = c // 2, c % 2
        full[b, _query_rows(p), :] = res.results[c]["out"]
    return full, res


def kernel(x, Wq, Wk, Wv):
    full, _ = run(x, Wq, Wk, Wv, trace=False)
    return full


if __name__ == "__main__":
    # quick smoke: build program only
    nc = build_program()
    print("program built ok")
